# revision 47
# baseline (speedup 1.0000x reference)
"""ROI max-pooling (B=2, N=64, C=256, H=W=64, 7x7 out) on 8 TRN2 cores — v4.

Design (v4):
- Host converts conv_out to fp16 and pre-slices a per-core row band
  (uniform R_MAX rows across cores); core = b*4 + nh*2 + cg.
- All reduction on the DVE, with forms chosen for the DVE perf modes:
  * TensorTensor max gets 2x_1p (0.5 cyc/elem) when all operands are
    fp16 with a packed (stride-1, count>=2) innermost AP dim.
  * TensorCopy gets 2x_2p (SBUF) / 4x when also packed.
  * TensorReduce gets NO perf mode (1 cyc/elem) — avoid where a TT
    chain is cheaper.
  Forms:
  * kh=1,kw=1: single 4x copy slab->ostage.
  * kh=1,kw=2: single TT from slab.
  * kh=1,kw>=3: single reduce from slab.
  * kh>=2,kw=1 (sw==1 always in-dist): TT chain writing ostage
    directly (wext==7), no tmp, no horizontal stage.
  * kh>=2,kw>=2: vertical TT chain into a [7,wext] fp16 strip (always
    2x), then a horizontal stage from the strip. Horizontal stages of
    two ROIs with the same (sw,kw) are PAIRED: both strips go in one
    tmp and one instruction covers both output slots.
- Same-engine RAW chains (vertical steps, chained horizontals) are
  spaced by interleaving the two chains of a pair and by flushing the
  previous pair's horizontal into the gap; engine_nop as last resort.
- Output staged fp16, slot order = completion order, slice marks are
  uniform across bodies so SP/Act need no registers or branches.
"""

import os

os.environ.setdefault("MYCRO_LOCAL_CACHE", "1")

import numpy as np

B, N, C, H, W = 2, 64, 256, 64, 64
POOL_H = POOL_W = 7
ANCHOR_STRIDE = 16
N_CORES = 8
N_PER_CORE = N // 2  # 32
CELLS = POOL_H * POOL_W  # 49
N_CHUNKS = 4

# ---- DVE cost model (ns), calibrated from HW traces ------------------------
F_OP = 72.0  # per-instruction fixed busy (TT/copy; reduce ~62)
R1 = 1.042  # cyc/elem, no perf mode (reduce, strided TT)
R2 = 0.53  # 2x_1p TT / 2x_2p copy
R4 = 0.53  # copy (4x not observed on HW; treat as 2x)

# schedule model constants (ns), relative to NEFF t=0
_ROW_NS = 108.0  # fp16 row (128p x 64 x 2B = 16 KiB) per-queue transfer
_T_SEM = 1050.0  # completion sem propagation + wake
_T_SETUP = 10600.0  # vector regs + branch resolved (abs)


def _wext(sw, kw):
    return 6 * sw + kw


def _roi_cost(kh, kw, sh, sw):
    """(cost_ns, n_ops) for one ROI, horizontal unpaired."""
    if kh == 1:
        if kw == 1:
            return (F_OP + R4 * 49, 1)
        if kw == 2:
            r = R2 if sw == 1 else R1
            return (F_OP + r * 49, 1)
        return (F_OP + R1 * 49 * kw, 1)
    if kw == 1:
        return ((kh - 1) * (F_OP + R2 * 49), kh - 1)
    we = _wext(sw, kw)
    v = (kh - 1) * (F_OP + R2 * 7 * we)
    h, nh = _h_cost(sw, kw, 1)
    return (v + h, kh - 1 + nh)


def _h_cost(sw, kw, npair):
    """(cost_ns, n_ops) of the horizontal stage covering npair strips."""
    e = 49 * npair
    if kw == 1:
        return (F_OP + R2 * e, 1)  # strided copy, 2x_2p
    if kw == 2:
        r = R2 if sw == 1 else R1
        return (F_OP + r * e, 1)
    if kw == 3:
        return (2 * (F_OP + R1 * e), 2)  # TT chain
    return (F_OP + R1 * e * kw, 1)  # kw=4: reduce


# ---- roi params ------------------------------------------------------------
def _expand(lo, hi, pool, limit):
    for _ in range(pool):
        need = (hi - lo + 1) < pool
        lo = np.where(need, np.maximum(0, lo - 1), lo)
        hi = np.where(need, np.minimum(limit - 1, hi + 1), hi)
    return lo, hi


def _roi_params(rois: np.ndarray):
    coords = (np.asarray(rois, np.float32) / ANCHOR_STRIDE).astype(np.int32)
    x1, y1, x2, y2 = (coords[..., i] for i in range(4))
    y1, y2 = _expand(y1, y2, POOL_H, H)
    x1, x2 = _expand(x1, x2, POOL_W, W)
    rh = y2 - y1 + 1
    rw = x2 - x1 + 1
    kh = -(-rh // POOL_H)
    sh = rh // POOL_H
    kw = -(-rw // POOL_W)
    sw = rw // POOL_W
    return y1, x1, sh, sw, kh, kw


# ---- planning --------------------------------------------------------------
def _row_extent(params, b, n):
    y1, x1, sh, sw, kh, kw = params
    lo = int(y1[b, n])
    hi = lo + 6 * int(sh[b, n]) + int(kh[b, n]) - 1
    return lo, hi


def _chunk_bounds(r_max, c0):
    rest = r_max - c0
    s1 = max(10, rest // 3)
    s2 = (rest - s1) // 2
    sizes = [c0, s1, s2, rest - s1 - s2]
    bounds = []
    acc = 0
    for s in sizes:
        acc += s
        bounds.append(acc)
    return bounds


def _c0_for(params, rs, metas, r_max):
    """Chunk-0 rows: enough that every body has >=1 ROI resident."""
    need = 0
    for rois, (bands, offs, rows) in zip(rs, metas):
        phis = sorted(
            offs[b] + _row_extent(params, b, n)[1] for b, n in rois
        )
        need = max(need, phis[0] + 1)
    return max(4, min(need, r_max // 2))


def _land_times(r_max, c0):
    """Modeled absolute landing time per chunk.
    q0 carries c0,c2 (first bytes ~10.1us); q1 carries c1,c3 (~10.6us)."""
    b = _chunk_bounds(r_max, c0)
    s = [b[0]] + [b[i] - b[i - 1] for i in range(1, N_CHUNKS)]
    return [
        10100 + s[0] * _ROW_NS + _T_SEM,
        10600 + s[1] * _ROW_NS + _T_SEM,
        10100 + (s[0] + s[2]) * _ROW_NS + _T_SEM,
        10600 + (s[1] + s[3]) * _ROW_NS + _T_SEM,
    ]


def _chunk_of(row, r_max, c0):
    bounds = _chunk_bounds(r_max, c0)
    rel = min(max(row, 0), r_max - 1)
    for c, e in enumerate(bounds):
        if rel < e:
            return c
    return N_CHUNKS - 1


def _bands(params, rois):
    bands = {}
    for b, n in rois:
        lo, hi = _row_extent(params, b, n)
        if b in bands:
            bands[b] = (min(bands[b][0], lo), max(bands[b][1], hi))
        else:
            bands[b] = (lo, hi)
    rows = 0
    offs = {}
    for b in sorted(bands):
        offs[b] = rows - bands[b][0]
        rows += bands[b][1] - bands[b][0] + 1
    return bands, offs, rows


def _units_for(params, rois, offs, r_max, c0):
    """Build scheduling units.

    Split ROIs (kh>=2, kw>=2) form units of 1-4 members grouped as
    "vpairs": two members with the same (sh, sw, kw) share each vertical
    TT via an extra [baseB-baseA, 2] AP dim; a unit's members share one
    horizontal op (same (sw, kw)). kw=1 chains and kh=1 ones pair the
    same way (shared ops with a pair dim on both src and dst).
    """
    y1, x1, sh, sw, kh, kw = params

    def cneed(bn):
        b, n = bn
        lo, hi = _row_extent(params, b, n)
        return _chunk_of(offs[b] + hi, r_max, c0)

    splits, chains, ones = [], {}, {}
    sgroups = {}
    for bn in rois:
        b, n = bn
        _kh, _kw, _sh, _sw = (
            int(kh[b, n]), int(kw[b, n]), int(sh[b, n]), int(sw[b, n]))
        if _kh >= 2 and _kw >= 2:
            sgroups.setdefault((_sh, _sw, _kw), []).append(bn)
        elif _kh >= 2:
            chains.setdefault(_sh, []).append(bn)
        else:
            ones.setdefault((_kw, _sh, _sw), []).append(bn)

    units = []

    # split rois: same-(sh,sw,kw) vpairs, then vpairs merged into units
    # of up to 4 rois with the same (sw,kw)
    vpairs = {}
    for (s_, w_, k_), mem in sgroups.items():
        mem.sort(key=lambda bn: (int(kh[bn[0], bn[1]]), cneed(bn)))
        i = 0
        while i < len(mem):
            if i + 1 < len(mem) and cneed(mem[i + 1]) <= cneed(mem[i]) + 1:
                vpairs.setdefault((w_, k_), []).append(mem[i : i + 2])
                i += 2
            else:
                vpairs.setdefault((w_, k_), []).append([mem[i]])
                i += 1
    for key, vps in vpairs.items():
        vps.sort(key=lambda vp: max(cneed(bn) for bn in vp))
        i = 0
        while i < len(vps):
            if (
                i + 1 < len(vps)
                and len(vps[i]) + len(vps[i + 1]) <= 4
                and max(cneed(bn) for bn in vps[i + 1])
                <= max(cneed(bn) for bn in vps[i]) + 1
            ):
                grp = [vps[i], vps[i + 1]]
                i += 2
            else:
                grp = [vps[i]]
                i += 1
            units.append(dict(kind="split", vpairs=grp, key=key,
                              rois=[bn for vp in grp for bn in vp]))

    # kw=1 chains: pair by same sh (kh may differ -> tail steps)
    for s_, mem in chains.items():
        mem.sort(key=lambda bn: (int(kh[bn[0], bn[1]]), cneed(bn)))
        i = 0
        while i < len(mem):
            if i + 1 < len(mem) and cneed(mem[i + 1]) <= cneed(mem[i]) + 1:
                units.append(dict(kind="chain", rois=mem[i : i + 2]))
                i += 2
            else:
                units.append(dict(kind="chain", rois=[mem[i]]))
                i += 1

    # kh=1 ones: pair by same (kw,sh,sw) -> one shared op
    for key1, mem in ones.items():
        mem.sort(key=cneed)
        i = 0
        while i < len(mem):
            if i + 1 < len(mem) and cneed(mem[i + 1]) <= cneed(mem[i]) + 1:
                units.append(dict(kind="one", rois=mem[i : i + 2]))
                i += 2
            else:
                units.append(dict(kind="one", rois=[mem[i]]))
                i += 1

    def sbase(bn):
        b, n = bn
        return (int(y1[b, n]) + offs[b]) * W + int(x1[b, n])

    for u in units:
        if u["kind"] == "split":
            for vp in u["vpairs"]:
                vp.sort(key=sbase)
            u["rois"] = [bn for vp in u["vpairs"] for bn in vp]
        else:
            u["rois"].sort(key=sbase)

    for u in units:
        u["cneed"] = max(cneed(bn) for bn in u["rois"])
        cost = 0.0
        if u["kind"] == "split":
            _sw, _kw = u["key"]
            we = _wext(_sw, _kw)
            for vp in u["vpairs"]:
                khs = sorted(int(kh[b, n]) for b, n in vp)
                if len(vp) == 2:
                    shared = khs[0] - 1
                    tail = khs[1] - khs[0]
                    cost += shared * (F_OP + R2 * 14 * we)
                    cost += tail * (F_OP + R2 * 7 * we)
                else:
                    cost += (khs[0] - 1) * (F_OP + R2 * 7 * we)
            cost += _h_cost(_sw, _kw, len(u["rois"]))[0]
        elif u["kind"] == "chain":
            khs = sorted(int(kh[b, n]) for b, n in u["rois"])
            if len(u["rois"]) == 2:
                cost += (khs[0] - 1) * (F_OP + R2 * 98)
                cost += (khs[1] - khs[0]) * (F_OP + R2 * 49)
            else:
                cost += (khs[0] - 1) * (F_OP + R2 * 49)
        else:
            b, n = u["rois"][0]
            _kw, _sw = int(kw[b, n]), int(sw[b, n])
            m = len(u["rois"])
            if _kw == 1:
                cost += F_OP + R4 * 49 * m
            elif _kw == 2:
                r = R2 if _sw == 1 else R1
                cost += F_OP + r * 49 * m
            else:
                cost += F_OP + R1 * 49 * _kw * m
        u["cost"] = cost
    return units


def _sched_units(units, r_max, c0):
    """Order units by chunk readiness; return (makespan, ordered units)."""
    land = _land_times(r_max, c0)
    units = sorted(units, key=lambda u: (u["cneed"], -u["cost"]))
    clk = _T_SETUP
    for u in units:
        clk = max(clk, land[u["cneed"]]) + u["cost"]
    return clk, units


def _plan(params):
    bodies_rois = []
    for b in range(B):
        ext = [_row_extent(params, b, n) for n in range(N)]
        order = sorted(range(N), key=lambda n: ext[n][0] + ext[n][1])
        bodies_rois.append([(b, n) for n in order[:N_PER_CORE]])
        bodies_rois.append([(b, n) for n in order[N_PER_CORE:]])

    def score(rs):
        metas = [_bands(params, r) for r in rs]
        r_used = max(m[2] for m in metas)
        r_used = min(2 * H, -(-r_used // 4) * 4)
        c0 = _c0_for(params, rs, metas, r_used)
        mks = []
        for r, (bands, offs, rows) in zip(rs, metas):
            units = _units_for(params, r, offs, r_used, c0)
            mk, _ = _sched_units(units, r_used, c0)
            mks.append(mk)
        return max(mks) + 3.0 * r_used + 8.0 * c0

    base_rois = [list(r) for r in bodies_rois]
    best_rois, best_score = None, None
    for seed in range(3):
        bodies_rois = [list(r) for r in base_rois]
        cur = score(bodies_rois)
        rng = np.random.default_rng(seed)
        pairs = [(0, 1), (2, 3)] * 6 + [(0, 2), (1, 3), (0, 3), (1, 2)]
        for it in range(2500):
            if it % 3 < 2:
                j1, j2 = pairs[int(rng.integers(0, len(pairs)))]
            else:
                j1, j2 = int(rng.integers(0, 4)), int(rng.integers(0, 4))
                if j1 == j2:
                    continue
            i1 = int(rng.integers(0, N_PER_CORE))
            i2 = int(rng.integers(0, N_PER_CORE))
            a, bq = bodies_rois[j1], bodies_rois[j2]
            a[i1], bq[i2] = bq[i2], a[i1]
            new = score(bodies_rois)
            if new <= cur:
                cur = new
            else:
                a[i1], bq[i2] = bq[i2], a[i1]
        if best_score is None or cur < best_score:
            best_rois, best_score = [list(r) for r in bodies_rois], cur
    bodies_rois = best_rois

    metas = [_bands(params, r) for r in bodies_rois]
    r_max = max(m[2] for m in metas)
    r_max = min(2 * H, -(-r_max // 4) * 4)
    c0 = _c0_for(params, bodies_rois, metas, r_max)

    bodies = []
    for j in range(4):
        rois = bodies_rois[j]
        bands, offs, rows = metas[j]
        units = _units_for(params, rois, offs, r_max, c0)
        mk, order = _sched_units(units, r_max, c0)
        # slots in completion order
        slots = {}
        s = 0
        for u in order:
            for bn in u["rois"]:
                slots[bn] = s
                s += 1
        bodies.append(
            dict(rois=rois, bands=bands, offs=offs, units=order, slots=slots, mk=mk)
        )
    return bodies, r_max, c0


# ---- device program --------------------------------------------------------
MARKS = [12, 24, 30, 32]  # uniform completion-count slice marks


def _build_nc(params):
    import contextlib

    import concourse.bass as bass
    from concourse import mybir

    y1, x1, sh, sw, kh, kw = params
    f16 = mybir.dt.float16

    bodies, r_max, c0 = _plan(params)
    FS = r_max * W
    OS = N_PER_CORE * CELLS
    bounds = _chunk_bounds(r_max, c0)
    starts = [0] + bounds[:-1]

    branch_order = sorted(range(4), key=lambda j: -bodies[j]["mk"])

    nc = bass.Bass(monotonic_sem_count=0)
    conv = nc.declare_dram_parameter("conv", [128, FS], f16, isOutput=False)
    out = nc.declare_dram_parameter("out", [128, OS], f16, isOutput=True)

    with contextlib.ExitStack() as ctx:
        slab = ctx.enter_context(nc.sbuf_tensor("slab", [128, FS], f16))
        ostage = ctx.enter_context(nc.sbuf_tensor("ostage", [128, OS], f16))
        tmps = [
            ctx.enter_context(nc.sbuf_tensor(f"tmp{i}", [128, 4 * 7 * 22], f16))
            for i in range(4)
        ]
        chunk_sems = [
            ctx.enter_context(nc.semaphore(f"chunk{c}")) for c in range(N_CHUNKS)
        ]
        vsem = ctx.enter_context(nc.semaphore("vsem"))
        osem = ctx.enter_context(nc.semaphore("osem"))
        block = ctx.enter_context(nc.Block())

        sl = slab[:]
        slab_t = sl.tensor
        part_pair = list(sl.ap[0])

        def chunk_dma(eng, c):
            eng.dma_start(
                slab[:, starts[c] * W : bounds[c] * W],
                conv[:, starts[c] * W : bounds[c] * W],
            ).then_inc(chunk_sems[c], 16)

        def out_slice(eng, lo_s, hi_s, thresh):
            eng.wait_ge(vsem, thresh)
            eng.dma_start(
                out[:, lo_s * CELLS : hi_s * CELLS],
                ostage[:, lo_s * CELLS : hi_s * CELLS],
            ).then_inc(osem, 16)

        @block.sync
        def _(sync):
            chunk_dma(sync, 0)
            chunk_dma(sync, 2)
            out_slice(sync, 0, MARKS[0], 1)
            out_slice(sync, MARKS[1], MARKS[2], 3)
            out_slice(sync, MARKS[2], MARKS[2] + 1, 4)

        @block.scalar
        def _(scalar):
            chunk_dma(scalar, 1)
            chunk_dma(scalar, 3)
            out_slice(scalar, MARKS[0], MARKS[1], 2)
            out_slice(scalar, MARKS[2] + 1, MARKS[3], 4)

        AluMax = mybir.AluOpType.max
        AxisX = mybir.AxisListType.X

        def emit_body(vector, j):
            # wrap compute methods to log (op, free_elems, packed) per emission
            dbg = DEBUG_OPS.setdefault(j, [])

            def _packed(ap):
                try:
                    last = ap.ap[-1]
                    return last[0] in (1, -1) and last[1] >= 2
                except Exception:
                    return False

            def _fs(ap):
                fs = 1
                for st, ct in list(ap.ap)[1:]:
                    fs *= ct
                return fs

            _tt, _rm, _tc, _nop = (vector.tensor_tensor, vector.reduce_max,
                                   vector.tensor_copy, vector.engine_nop)
            _dma = vector.dma_start

            def tt(out, a, b_, **kw_):
                dbg.append(("tt", max(_fs(a), _fs(b_), _fs(out)),
                            _packed(a) and _packed(b_) and _packed(out)))
                return _tt(out, a, b_, **kw_)

            def rm(out, in_, **kw_):
                dbg.append(("red", max(_fs(in_), _fs(out)), False))
                return _rm(out, in_, **kw_)

            def tc(out, in_):
                dbg.append(("copy", max(_fs(in_), _fs(out)),
                            _packed(in_) and _packed(out)))
                return _tc(out, in_)

            def nop():
                dbg.append(("nop", 0, False))
                return _nop()

            vector = type("V", (), dict(
                tensor_tensor=staticmethod(tt), reduce_max=staticmethod(rm),
                tensor_copy=staticmethod(tc), engine_nop=staticmethod(nop),
                dma_start=staticmethod(_dma),
                wait_ge=staticmethod(vector.wait_ge)))()

            bd = bodies[j]
            offs = bd["offs"]
            slots = bd["slots"]
            units = bd["units"]

            def slab_ap(b, n, dr, dc, inner):
                base = (
                    sl.offset
                    + (int(y1[b, n]) + offs[b] + dr) * W
                    + int(x1[b, n])
                    + dc
                )
                return bass.AP(slab_t, base, [part_pair] + inner)

            def slot_ap(s, count=1):
                return ostage[:, s * CELLS : (s + count) * CELLS]

            waited = set()
            done = 0
            mark_i = 0
            last_chain = [None]  # chain id of previously emitted op
            pending = []  # list of (chain_id, emit_fn, completes)

            def emit(chain_id, fn, completes=()):
                """Emit one op; flush pending H ops into gaps."""
                nonlocal done, mark_i
                inst = fn()
                last_chain[0] = chain_id
                _complete(inst, completes)
                return inst

            def _complete(inst, completes):
                nonlocal done, mark_i
                if not completes:
                    return
                done += len(completes)
                incs = 0
                while mark_i < len(MARKS) and done >= MARKS[mark_i]:
                    incs += 1
                    mark_i += 1
                if incs:
                    inst.then_inc(vsem, incs)

            def flush_pending(force=False):
                """Emit pending H ops whose dep chain differs from last op."""
                while pending:
                    cid, fn, comps = pending[0]
                    if cid == last_chain[0]:
                        if not force:
                            return
                        vector.engine_nop()
                        last_chain[0] = None
                    pending.pop(0)
                    inst = fn()
                    last_chain[0] = cid
                    _complete(inst, comps)

            def pair_dims(mem, inner):
                """AP dims: leading [baseB-baseA, 2] when mem has 2 rois."""
                if len(mem) == 2:
                    d0 = sbase(mem[0])
                    d1 = sbase(mem[1])
                    return [[d1 - d0, 2]] + inner
                return list(inner)

            def sbase(bn):
                b, n = bn
                return (int(y1[b, n]) + offs[b]) * W + int(x1[b, n])

            def grp_ap(mem, dr, dc, inner):
                b, n = mem[0]
                base = sl.offset + sbase(mem[0]) + dr * W + dc
                return bass.AP(slab_t, base, [part_pair] + pair_dims(mem, inner))

            tmp_i = 0
            for u in units:
                for cc in range(u["cneed"] + 1):
                    if cc not in waited:
                        vector.wait_ge(chunk_sems[cc], 16)
                        waited.add(cc)
                kind = u["kind"]
                mem = u["rois"]
                if kind == "one":
                    b, n = mem[0]
                    _kw, _sh, _sw = (
                        int(kw[b, n]), int(sh[b, n]), int(sw[b, n]))
                    m = len(mem)
                    s0 = slots[mem[0]]
                    assert [slots[bn] for bn in mem] == list(range(s0, s0 + m))
                    dst = slot_ap(s0, m)
                    inner = [[_sh * W, 7], [_sw, 7]]
                    cid = mem[-1]
                    if _kw == 1:
                        emit(cid, lambda dst=dst, mem=mem, inner=inner:
                             vector.tensor_copy(dst, grp_ap(mem, 0, 0, inner)),
                             list(mem))
                    elif _kw == 2:
                        emit(cid, lambda dst=dst, mem=mem, inner=inner:
                             vector.tensor_tensor(
                                 dst, grp_ap(mem, 0, 0, inner),
                                 grp_ap(mem, 0, 1, inner), op=AluMax),
                             list(mem))
                    else:
                        emit(cid, lambda dst=dst, mem=mem, inner=inner, _kw=_kw:
                             vector.reduce_max(
                                 dst, grp_ap(mem, 0, 0, inner + [[1, _kw]]),
                                 axis=AxisX),
                             list(mem))
                    flush_pending()
                elif kind == "chain":
                    khs = [int(kh[b, n]) for b, n in mem]
                    _sh = int(sh[mem[0][0], mem[0][1]])
                    m = len(mem)
                    s0 = slots[mem[0]]
                    assert [slots[bn] for bn in mem] == list(range(s0, s0 + m))
                    dst = slot_ap(s0, m)
                    inner = [[_sh * W, 7], [1, 7]]
                    cid = mem[-1]
                    khmin, khmax = min(khs), max(khs)
                    emit(cid, lambda dst=dst, mem=mem, inner=inner:
                         vector.tensor_tensor(
                             dst, grp_ap(mem, 0, 0, inner),
                             grp_ap(mem, 1, 0, inner), op=AluMax),
                         [bn for bn, k_ in zip(mem, khs) if k_ == 2])
                    for d in range(2, khmax):
                        cur = [bn for bn, k_ in zip(mem, khs) if k_ > d]
                        cd = dst if len(cur) == m else \
                            slot_ap(slots[cur[0]], len(cur))
                        if last_chain[0] == cid:
                            flush_pending()
                        if last_chain[0] == cid:
                            vector.engine_nop()
                            last_chain[0] = None
                        comps = [bn for bn, k_ in zip(mem, khs) if k_ == d + 1]
                        emit(cid, lambda cd=cd, cur=cur, inner=inner, d=d:
                             vector.tensor_tensor(
                                 cd, cd, grp_ap(cur, d, 0, inner), op=AluMax),
                             comps)
                    flush_pending()
                else:  # split
                    _sw, _kw = u["key"]
                    we = _wext(_sw, _kw)
                    tmp = tmps[tmp_i % 4]
                    tmp_i += 1
                    vpairs = u["vpairs"]
                    tap = tmp[:]
                    tpart = list(tap.ap[0])
                    # strip APs per vpair (pair dim over the 2 strips)
                    off_i = {}
                    idx0 = 0
                    for vp in vpairs:
                        off_i[id(vp)] = idx0
                        idx0 += len(vp)

                    def strip_ap(vp, solo_idx=None):
                        i0 = off_i[id(vp)]
                        if solo_idx is not None:
                            o = tap.offset + (i0 + solo_idx) * 7 * we
                            return bass.AP(tap.tensor, o,
                                           [tpart, [we, 7], [1, we]])
                        if len(vp) == 2:
                            return bass.AP(tap.tensor, tap.offset + i0 * 7 * we,
                                           [tpart, [7 * we, 2], [we, 7], [1, we]])
                        return bass.AP(tap.tensor, tap.offset + i0 * 7 * we,
                                       [tpart, [we, 7], [1, we]])

                    khmax_all = max(int(kh[b, n]) for b, n in mem)
                    for d in range(1, khmax_all):
                        for vp in vpairs:
                            khs = [int(kh[b, n]) for b, n in vp]
                            _shv = int(sh[vp[0][0], vp[0][1]])
                            inner = [[_shv * W, 7], [1, we]]
                            cidv = ("v",) + tuple(vp)
                            if len(vp) == 2 and d < min(khs):
                                sap = strip_ap(vp)
                                if d == 1:
                                    fn = (lambda sap=sap, vp=vp, inner=inner:
                                          vector.tensor_tensor(
                                              sap, grp_ap(vp, 0, 0, inner),
                                              grp_ap(vp, 1, 0, inner),
                                              op=AluMax))
                                else:
                                    fn = (lambda sap=sap, vp=vp, inner=inner,
                                          d=d:
                                          vector.tensor_tensor(
                                              sap, sap,
                                              grp_ap(vp, d, 0, inner),
                                              op=AluMax))
                            else:
                                # solo step for whichever members still run
                                cur = [(i3, bn) for i3, (bn, k_) in
                                       enumerate(zip(vp, khs)) if k_ > d]
                                if not cur:
                                    continue
                                assert len(cur) == 1
                                i3, bn = cur[0]
                                sap = strip_ap(vp, solo_idx=i3)
                                if d == 1:
                                    fn = (lambda sap=sap, bn=bn, inner=inner:
                                          vector.tensor_tensor(
                                              sap, grp_ap([bn], 0, 0, inner),
                                              grp_ap([bn], 1, 0, inner),
                                              op=AluMax))
                                else:
                                    fn = (lambda sap=sap, bn=bn, inner=inner,
                                          d=d:
                                          vector.tensor_tensor(
                                              sap, sap,
                                              grp_ap([bn], d, 0, inner),
                                              op=AluMax))
                            if last_chain[0] == cidv:
                                flush_pending()
                            if last_chain[0] == cidv:
                                vector.engine_nop()
                                last_chain[0] = None
                            emit(cidv, fn)
                        if d == 1:
                            flush_pending()
                    # horizontal over all strips
                    npair = len(mem)
                    s0 = min(slots[bn] for bn in mem)
                    assert [slots[bn] for bn in mem] == list(
                        range(s0, s0 + npair))
                    hcid = last_chain[0]

                    def mk_hsrc(tap_t=tap.tensor, tap_off=tap.offset,
                                tpart=tuple(tpart), we=we, sw_=_sw,
                                npair=npair):
                        def hsrc(dc, extra=None):
                            dims = ([[7 * we, npair]] if npair >= 2 else []) \
                                + [[we, 7], [sw_, 7]]
                            if extra:
                                dims = dims + [extra]
                            return bass.AP(tap_t, tap_off + dc,
                                           [list(tpart)] + dims)
                        return hsrc

                    hsrc = mk_hsrc()
                    out2 = slot_ap(s0, npair)
                    if _kw == 1:
                        pending.append((hcid, lambda out2=out2, hsrc=hsrc:
                                        vector.tensor_copy(out2, hsrc(0)),
                                        list(mem)))
                    elif _kw == 2:
                        pending.append((hcid, lambda out2=out2, hsrc=hsrc:
                                        vector.tensor_tensor(
                                            out2, hsrc(0), hsrc(1), op=AluMax),
                                        list(mem)))
                    elif _kw == 3:
                        pending.append((hcid, lambda out2=out2, hsrc=hsrc:
                                        vector.tensor_tensor(
                                            out2, hsrc(0), hsrc(1), op=AluMax),
                                        ()))
                        pending.append((hcid, lambda out2=out2, hsrc=hsrc:
                                        vector.tensor_tensor(
                                            out2, out2, hsrc(2), op=AluMax),
                                        list(mem)))
                    else:
                        pending.append((hcid, lambda out2=out2, hsrc=hsrc,
                                        _kw=_kw:
                                        vector.reduce_max(
                                            out2, hsrc(0, [1, _kw]), axis=AxisX),
                                        list(mem)))
            flush_pending(force=True)
            if mark_i < len(MARKS):
                raise RuntimeError("marks not all reached")

        @block.vector
        def _(vector):
            pid = vector.alloc_register("pid")
            vector.reg_load(pid, nc.partition_id_tensor[0:1, 0:1])

            # balanced dispatch: 2 branches deep for every core
            with vector.If_lt(pid, 4):
                with vector.If_lt(pid, 2):
                    emit_body(vector, 0)
                with vector.Else():
                    emit_body(vector, 1)
            with vector.Else():
                with vector.If_lt(pid, 6):
                    emit_body(vector, 2)
                with vector.Else():
                    emit_body(vector, 3)

    return nc, bodies, r_max


_CACHE: dict[bytes, object] = {}
LAST_RESULT = None
LAST_PLAN = None
DEBUG_OPS: dict[int, list] = {}


def _get_built(params_key: bytes, params):
    built = _CACHE.get(params_key)
    if built is None:
        built = _build_nc(params)
        _CACHE[params_key] = built
    return built


def kernel(rois: np.ndarray, conv_out: np.ndarray) -> np.ndarray:
    from concourse.bass_utils import run_bass_kernel_spmd

    rois = np.asarray(rois)
    conv_out = np.asarray(conv_out, np.float32)
    params = _roi_params(rois)
    params_key = b"".join(np.ascontiguousarray(p).tobytes() for p in params)
    nc, bodies, r_max = _get_built(params_key, params)
    global LAST_PLAN
    LAST_PLAN = (bodies, r_max)

    in_maps = []
    for core in range(N_CORES):
        j, cg = core >> 1, core & 1
        bd = bodies[j]
        slab = np.zeros((128, r_max, W), np.float16)
        for b, (lo, hi) in bd["bands"].items():
            off = bd["offs"][b] + lo
            slab[:, off : off + hi - lo + 1] = conv_out[
                b, cg * 128 : (cg + 1) * 128, lo : hi + 1, :
            ]
        in_maps.append({"conv": slab.reshape(128, -1)})

    res = run_bass_kernel_spmd(nc, in_maps, list(range(N_CORES)))
    global LAST_RESULT
    LAST_RESULT = res

    out = np.empty((B, N, C, POOL_H, POOL_W), np.float32)
    for core in range(N_CORES):
        j, cg = core >> 1, core & 1
        bd = bodies[j]
        r = (
            res.results[core]["out"]
            .reshape(128, N_PER_CORE, CELLS)
            .astype(np.float32)
        )
        for b, n in bd["rois"]:
            s = bd["slots"][(b, n)]
            out[b, n, cg * 128 : (cg + 1) * 128] = r[:, s].reshape(
                128, POOL_H, POOL_W
            )
    return out


# revision 49
# speedup vs baseline: 1.0099x; 1.0099x over previous
"""ROI max-pooling (B=2, N=64, C=256, H=W=64, 7x7 out) on 8 TRN2 cores — v4.

Design (v4):
- Host converts conv_out to fp16 and pre-slices a per-core row band
  (uniform R_MAX rows across cores); core = b*4 + nh*2 + cg.
- All reduction on the DVE, with forms chosen for the DVE perf modes:
  * TensorTensor max gets 2x_1p (0.5 cyc/elem) when all operands are
    fp16 with a packed (stride-1, count>=2) innermost AP dim.
  * TensorCopy gets 2x_2p (SBUF) / 4x when also packed.
  * TensorReduce gets NO perf mode (1 cyc/elem) — avoid where a TT
    chain is cheaper.
  Forms:
  * kh=1,kw=1: single 4x copy slab->ostage.
  * kh=1,kw=2: single TT from slab.
  * kh=1,kw>=3: single reduce from slab.
  * kh>=2,kw=1 (sw==1 always in-dist): TT chain writing ostage
    directly (wext==7), no tmp, no horizontal stage.
  * kh>=2,kw>=2: vertical TT chain into a [7,wext] fp16 strip (always
    2x), then a horizontal stage from the strip. Horizontal stages of
    two ROIs with the same (sw,kw) are PAIRED: both strips go in one
    tmp and one instruction covers both output slots.
- Same-engine RAW chains (vertical steps, chained horizontals) are
  spaced by interleaving the two chains of a pair and by flushing the
  previous pair's horizontal into the gap; engine_nop as last resort.
- Output staged fp16, slot order = completion order, slice marks are
  uniform across bodies so SP/Act need no registers or branches.
"""

import os

os.environ.setdefault("MYCRO_LOCAL_CACHE", "1")

import numpy as np

B, N, C, H, W = 2, 64, 256, 64, 64
POOL_H = POOL_W = 7
ANCHOR_STRIDE = 16
N_CORES = 8
N_PER_CORE = N // 2  # 32
CELLS = POOL_H * POOL_W  # 49
N_CHUNKS = 4

# ---- DVE cost model (ns), calibrated from HW traces ------------------------
F_OP = 72.0  # per-instruction fixed busy (TT/copy; reduce ~62)
R1 = 1.042  # cyc/elem, no perf mode (reduce, strided TT)
R2 = 0.53  # 2x_1p TT / 2x_2p copy
R4 = 0.53  # copy (4x not observed on HW; treat as 2x)

# schedule model constants (ns), relative to NEFF t=0
_ROW_NS = 108.0  # fp16 row (128p x 64 x 2B = 16 KiB) per-queue transfer
_T_SEM = 1050.0  # completion sem propagation + wake
_T_SETUP = 10600.0  # vector regs + branch resolved (abs)


def _wext(sw, kw):
    return 6 * sw + kw


def _roi_cost(kh, kw, sh, sw):
    """(cost_ns, n_ops) for one ROI, horizontal unpaired."""
    if kh == 1:
        if kw == 1:
            return (F_OP + R4 * 49, 1)
        if kw == 2:
            r = R2 if sw == 1 else R1
            return (F_OP + r * 49, 1)
        return (F_OP + R1 * 49 * kw, 1)
    if kw == 1:
        return ((kh - 1) * (F_OP + R2 * 49), kh - 1)
    we = _wext(sw, kw)
    v = (kh - 1) * (F_OP + R2 * 7 * we)
    h, nh = _h_cost(sw, kw, 1)
    return (v + h, kh - 1 + nh)


def _h_cost(sw, kw, npair):
    """(cost_ns, n_ops) of the horizontal stage covering npair strips."""
    e = 49 * npair
    if kw == 1:
        return (F_OP + R2 * e, 1)  # strided copy, 2x_2p
    if kw == 2:
        r = R2 if sw == 1 else R1
        return (F_OP + r * e, 1)
    if kw == 3:
        return (2 * (F_OP + R1 * e), 2)  # TT chain
    return (F_OP + R1 * e * kw, 1)  # kw=4: reduce


# ---- roi params ------------------------------------------------------------
def _expand(lo, hi, pool, limit):
    for _ in range(pool):
        need = (hi - lo + 1) < pool
        lo = np.where(need, np.maximum(0, lo - 1), lo)
        hi = np.where(need, np.minimum(limit - 1, hi + 1), hi)
    return lo, hi


def _roi_params(rois: np.ndarray):
    coords = (np.asarray(rois, np.float32) / ANCHOR_STRIDE).astype(np.int32)
    x1, y1, x2, y2 = (coords[..., i] for i in range(4))
    y1, y2 = _expand(y1, y2, POOL_H, H)
    x1, x2 = _expand(x1, x2, POOL_W, W)
    rh = y2 - y1 + 1
    rw = x2 - x1 + 1
    kh = -(-rh // POOL_H)
    sh = rh // POOL_H
    kw = -(-rw // POOL_W)
    sw = rw // POOL_W
    return y1, x1, sh, sw, kh, kw


# ---- planning --------------------------------------------------------------
def _row_extent(params, b, n):
    y1, x1, sh, sw, kh, kw = params
    lo = int(y1[b, n])
    hi = lo + 6 * int(sh[b, n]) + int(kh[b, n]) - 1
    return lo, hi


def _chunk_bounds(r_max, c0):
    rest = r_max - c0
    s1 = max(10, rest // 3)
    s2 = (rest - s1) // 2
    sizes = [c0, s1, s2, rest - s1 - s2]
    bounds = []
    acc = 0
    for s in sizes:
        acc += s
        bounds.append(acc)
    return bounds


def _c0_for(params, rs, metas, r_max):
    """Chunk-0 rows: enough that every body has >=1 ROI resident."""
    need = 0
    for rois, (bands, offs, rows) in zip(rs, metas):
        phis = sorted(
            offs[b] + _row_extent(params, b, n)[1] for b, n in rois
        )
        need = max(need, phis[0] + 1)
    return max(4, min(need, r_max // 2))


def _land_times(r_max, c0):
    """Modeled absolute landing time per chunk.
    q0 carries c0,c2 (first bytes ~10.1us); q1 carries c1,c3 (~10.6us)."""
    b = _chunk_bounds(r_max, c0)
    s = [b[0]] + [b[i] - b[i - 1] for i in range(1, N_CHUNKS)]
    return [
        10100 + s[0] * _ROW_NS + _T_SEM,
        10600 + s[1] * _ROW_NS + _T_SEM,
        10100 + (s[0] + s[2]) * _ROW_NS + _T_SEM,
        10600 + (s[1] + s[3]) * _ROW_NS + _T_SEM,
    ]


def _chunk_of(row, r_max, c0):
    bounds = _chunk_bounds(r_max, c0)
    rel = min(max(row, 0), r_max - 1)
    for c, e in enumerate(bounds):
        if rel < e:
            return c
    return N_CHUNKS - 1


def _bands(params, rois):
    bands = {}
    for b, n in rois:
        lo, hi = _row_extent(params, b, n)
        if b in bands:
            bands[b] = (min(bands[b][0], lo), max(bands[b][1], hi))
        else:
            bands[b] = (lo, hi)
    rows = 0
    offs = {}
    for b in sorted(bands):
        offs[b] = rows - bands[b][0]
        rows += bands[b][1] - bands[b][0] + 1
    return bands, offs, rows


def _units_for(params, rois, offs, r_max, c0):
    """Build scheduling units.

    Split ROIs (kh>=2, kw>=2) form units of 1-4 members grouped as
    "vpairs": two members with the same (sh, sw, kw) share each vertical
    TT via an extra [baseB-baseA, 2] AP dim; a unit's members share one
    horizontal op (same (sw, kw)). kw=1 chains and kh=1 ones pair the
    same way (shared ops with a pair dim on both src and dst).
    """
    y1, x1, sh, sw, kh, kw = params

    def cneed(bn):
        b, n = bn
        lo, hi = _row_extent(params, b, n)
        return _chunk_of(offs[b] + hi, r_max, c0)

    splits, chains, ones = [], {}, {}
    sgroups = {}
    for bn in rois:
        b, n = bn
        _kh, _kw, _sh, _sw = (
            int(kh[b, n]), int(kw[b, n]), int(sh[b, n]), int(sw[b, n]))
        if _kh >= 2 and _kw >= 2:
            sgroups.setdefault((_sh, _sw, _kw), []).append(bn)
        elif _kh >= 2:
            chains.setdefault(_sh, []).append(bn)
        else:
            ones.setdefault((_kw, _sh, _sw), []).append(bn)

    units = []

    # split rois: same-(sh,sw,kw) vpairs, then vpairs merged into units
    # of up to 4 rois with the same (sw,kw)
    vpairs = {}
    for (s_, w_, k_), mem in sgroups.items():
        mem.sort(key=lambda bn: (int(kh[bn[0], bn[1]]), cneed(bn)))
        i = 0
        while i < len(mem):
            if i + 1 < len(mem) and cneed(mem[i + 1]) <= cneed(mem[i]) + 1:
                vpairs.setdefault((w_, k_), []).append(mem[i : i + 2])
                i += 2
            else:
                vpairs.setdefault((w_, k_), []).append([mem[i]])
                i += 1
    for key, vps in vpairs.items():
        vps.sort(key=lambda vp: max(cneed(bn) for bn in vp))
        i = 0
        while i < len(vps):
            if (
                i + 1 < len(vps)
                and len(vps[i]) + len(vps[i + 1]) <= 4
                and max(cneed(bn) for bn in vps[i + 1])
                <= max(cneed(bn) for bn in vps[i]) + 1
            ):
                grp = [vps[i], vps[i + 1]]
                i += 2
            else:
                grp = [vps[i]]
                i += 1
            units.append(dict(kind="split", vpairs=grp, key=key,
                              rois=[bn for vp in grp for bn in vp]))

    # kw=1 chains: pair by same sh (kh may differ -> tail steps)
    for s_, mem in chains.items():
        mem.sort(key=lambda bn: (int(kh[bn[0], bn[1]]), cneed(bn)))
        i = 0
        while i < len(mem):
            if i + 1 < len(mem) and cneed(mem[i + 1]) <= cneed(mem[i]) + 1:
                units.append(dict(kind="chain", rois=mem[i : i + 2]))
                i += 2
            else:
                units.append(dict(kind="chain", rois=[mem[i]]))
                i += 1

    # kh=1 ones: pair by same (kw,sh,sw) -> one shared op
    for key1, mem in ones.items():
        mem.sort(key=cneed)
        i = 0
        while i < len(mem):
            if i + 1 < len(mem) and cneed(mem[i + 1]) <= cneed(mem[i]) + 1:
                units.append(dict(kind="one", rois=mem[i : i + 2]))
                i += 2
            else:
                units.append(dict(kind="one", rois=[mem[i]]))
                i += 1

    def sbase(bn):
        b, n = bn
        return (int(y1[b, n]) + offs[b]) * W + int(x1[b, n])

    for u in units:
        if u["kind"] == "split":
            for vp in u["vpairs"]:
                vp.sort(key=sbase)
            u["rois"] = [bn for vp in u["vpairs"] for bn in vp]
        else:
            u["rois"].sort(key=sbase)

    for u in units:
        u["cneed"] = max(cneed(bn) for bn in u["rois"])
        cost = 0.0
        if u["kind"] == "split":
            _sw, _kw = u["key"]
            we = _wext(_sw, _kw)
            for vp in u["vpairs"]:
                khs = sorted(int(kh[b, n]) for b, n in vp)
                if len(vp) == 2:
                    shared = khs[0] - 1
                    tail = khs[1] - khs[0]
                    cost += shared * (F_OP + R2 * 14 * we)
                    cost += tail * (F_OP + R2 * 7 * we)
                else:
                    cost += (khs[0] - 1) * (F_OP + R2 * 7 * we)
            cost += _h_cost(_sw, _kw, len(u["rois"]))[0]
        elif u["kind"] == "chain":
            khs = sorted(int(kh[b, n]) for b, n in u["rois"])
            if len(u["rois"]) == 2:
                cost += (khs[0] - 1) * (F_OP + R2 * 98)
                cost += (khs[1] - khs[0]) * (F_OP + R2 * 49)
            else:
                cost += (khs[0] - 1) * (F_OP + R2 * 49)
        else:
            b, n = u["rois"][0]
            _kw, _sw = int(kw[b, n]), int(sw[b, n])
            m = len(u["rois"])
            if _kw == 1:
                cost += F_OP + R4 * 49 * m
            elif _kw == 2:
                r = R2 if _sw == 1 else R1
                cost += F_OP + r * 49 * m
            else:
                cost += F_OP + R1 * 49 * _kw * m
        u["cost"] = cost
    return units


def _sched_units(units, r_max, c0):
    """Order units by chunk readiness; return (makespan, ordered units)."""
    land = _land_times(r_max, c0)
    units = sorted(units, key=lambda u: (u["cneed"], -u["cost"]))
    clk = _T_SETUP
    for u in units:
        clk = max(clk, land[u["cneed"]]) + u["cost"]
    return clk, units


def _plan(params):
    bodies_rois = []
    for b in range(B):
        ext = [_row_extent(params, b, n) for n in range(N)]
        order = sorted(range(N), key=lambda n: ext[n][0] + ext[n][1])
        bodies_rois.append([(b, n) for n in order[:N_PER_CORE]])
        bodies_rois.append([(b, n) for n in order[N_PER_CORE:]])

    def score(rs):
        metas = [_bands(params, r) for r in rs]
        r_used = max(m[2] for m in metas)
        r_used = min(2 * H, -(-r_used // 4) * 4)
        c0 = _c0_for(params, rs, metas, r_used)
        mks = []
        for r, (bands, offs, rows) in zip(rs, metas):
            units = _units_for(params, r, offs, r_used, c0)
            mk, _ = _sched_units(units, r_used, c0)
            mks.append(mk)
        return max(mks) + 3.0 * r_used + 8.0 * c0

    base_rois = [list(r) for r in bodies_rois]
    best_rois, best_score = None, None
    for seed in range(3):
        bodies_rois = [list(r) for r in base_rois]
        cur = score(bodies_rois)
        rng = np.random.default_rng(seed)
        pairs = [(0, 1), (2, 3)] * 6 + [(0, 2), (1, 3), (0, 3), (1, 2)]
        for it in range(2500):
            if it % 3 < 2:
                j1, j2 = pairs[int(rng.integers(0, len(pairs)))]
            else:
                j1, j2 = int(rng.integers(0, 4)), int(rng.integers(0, 4))
                if j1 == j2:
                    continue
            i1 = int(rng.integers(0, N_PER_CORE))
            i2 = int(rng.integers(0, N_PER_CORE))
            a, bq = bodies_rois[j1], bodies_rois[j2]
            a[i1], bq[i2] = bq[i2], a[i1]
            new = score(bodies_rois)
            if new <= cur:
                cur = new
            else:
                a[i1], bq[i2] = bq[i2], a[i1]
        if best_score is None or cur < best_score:
            best_rois, best_score = [list(r) for r in bodies_rois], cur
    bodies_rois = best_rois

    metas = [_bands(params, r) for r in bodies_rois]
    r_max = max(m[2] for m in metas)
    r_max = min(2 * H, -(-r_max // 4) * 4)
    c0 = _c0_for(params, bodies_rois, metas, r_max)

    bodies = []
    for j in range(4):
        rois = bodies_rois[j]
        bands, offs, rows = metas[j]
        units = _units_for(params, rois, offs, r_max, c0)
        mk, order = _sched_units(units, r_max, c0)
        # slots in completion order
        slots = {}
        s = 0
        for u in order:
            for bn in u["rois"]:
                slots[bn] = s
                s += 1
        bodies.append(
            dict(rois=rois, bands=bands, offs=offs, units=order, slots=slots, mk=mk)
        )
    return bodies, r_max, c0


# ---- device program --------------------------------------------------------
MARKS = [10, 20, 26, 32]  # uniform completion-count slice marks


def _build_nc(params):
    import contextlib

    import concourse.bass as bass
    from concourse import mybir

    y1, x1, sh, sw, kh, kw = params
    f16 = mybir.dt.float16

    bodies, r_max, c0 = _plan(params)
    FS = r_max * W
    OS = N_PER_CORE * CELLS
    bounds = _chunk_bounds(r_max, c0)
    starts = [0] + bounds[:-1]

    branch_order = sorted(range(4), key=lambda j: -bodies[j]["mk"])

    nc = bass.Bass(monotonic_sem_count=0)
    conv = nc.declare_dram_parameter("conv", [128, FS], f16, isOutput=False)
    out = nc.declare_dram_parameter("out", [128, OS], f16, isOutput=True)

    with contextlib.ExitStack() as ctx:
        slab = ctx.enter_context(nc.sbuf_tensor("slab", [128, FS], f16))
        ostage = ctx.enter_context(nc.sbuf_tensor("ostage", [128, OS], f16))
        tmps = [
            ctx.enter_context(nc.sbuf_tensor(f"tmp{i}", [128, 4 * 7 * 22], f16))
            for i in range(4)
        ]
        chunk_sems = [
            ctx.enter_context(nc.semaphore(f"chunk{c}")) for c in range(N_CHUNKS)
        ]
        vsem = ctx.enter_context(nc.semaphore("vsem"))
        osem = ctx.enter_context(nc.semaphore("osem"))
        block = ctx.enter_context(nc.Block())

        sl = slab[:]
        slab_t = sl.tensor
        part_pair = list(sl.ap[0])

        def chunk_dma(eng, c):
            eng.dma_start(
                slab[:, starts[c] * W : bounds[c] * W],
                conv[:, starts[c] * W : bounds[c] * W],
            ).then_inc(chunk_sems[c], 16)

        def out_slice(eng, lo_s, hi_s, thresh):
            eng.wait_ge(vsem, thresh)
            eng.dma_start(
                out[:, lo_s * CELLS : hi_s * CELLS],
                ostage[:, lo_s * CELLS : hi_s * CELLS],
            ).then_inc(osem, 16)

        @block.sync
        def _(sync):
            chunk_dma(sync, 0)
            chunk_dma(sync, 2)
            out_slice(sync, 0, MARKS[0], 1)
            out_slice(sync, MARKS[1], MARKS[2], 3)
            out_slice(sync, MARKS[2], MARKS[2] + 3, 4)

        @block.scalar
        def _(scalar):
            chunk_dma(scalar, 1)
            chunk_dma(scalar, 3)
            out_slice(scalar, MARKS[0], MARKS[1], 2)
            out_slice(scalar, MARKS[2] + 3, MARKS[3], 4)

        AluMax = mybir.AluOpType.max
        AxisX = mybir.AxisListType.X

        def emit_body(vector, j):
            # wrap compute methods to log (op, free_elems, packed) per emission
            dbg = DEBUG_OPS.setdefault(j, [])

            def _packed(ap):
                try:
                    last = ap.ap[-1]
                    return last[0] in (1, -1) and last[1] >= 2
                except Exception:
                    return False

            def _fs(ap):
                fs = 1
                for st, ct in list(ap.ap)[1:]:
                    fs *= ct
                return fs

            _tt, _rm, _tc, _nop = (vector.tensor_tensor, vector.reduce_max,
                                   vector.tensor_copy, vector.engine_nop)
            _dma = vector.dma_start

            def tt(out, a, b_, **kw_):
                dbg.append(("tt", max(_fs(a), _fs(b_), _fs(out)),
                            _packed(a) and _packed(b_) and _packed(out)))
                return _tt(out, a, b_, **kw_)

            def rm(out, in_, **kw_):
                dbg.append(("red", max(_fs(in_), _fs(out)), False))
                return _rm(out, in_, **kw_)

            def tc(out, in_):
                dbg.append(("copy", max(_fs(in_), _fs(out)),
                            _packed(in_) and _packed(out)))
                return _tc(out, in_)

            def nop():
                dbg.append(("nop", 0, False))
                return _nop()

            vector = type("V", (), dict(
                tensor_tensor=staticmethod(tt), reduce_max=staticmethod(rm),
                tensor_copy=staticmethod(tc), engine_nop=staticmethod(nop),
                dma_start=staticmethod(_dma),
                wait_ge=staticmethod(vector.wait_ge)))()

            bd = bodies[j]
            offs = bd["offs"]
            slots = bd["slots"]
            units = bd["units"]

            def slab_ap(b, n, dr, dc, inner):
                base = (
                    sl.offset
                    + (int(y1[b, n]) + offs[b] + dr) * W
                    + int(x1[b, n])
                    + dc
                )
                return bass.AP(slab_t, base, [part_pair] + inner)

            def slot_ap(s, count=1):
                return ostage[:, s * CELLS : (s + count) * CELLS]

            waited = set()
            done = 0
            mark_i = 0
            last_chain = [None]  # chain id of previously emitted op
            pending = []  # list of (chain_id, emit_fn, completes)

            def emit(chain_id, fn, completes=()):
                """Emit one op; flush pending H ops into gaps."""
                nonlocal done, mark_i
                inst = fn()
                last_chain[0] = chain_id
                _complete(inst, completes)
                return inst

            def _complete(inst, completes):
                nonlocal done, mark_i
                if not completes:
                    return
                done += len(completes)
                incs = 0
                while mark_i < len(MARKS) and done >= MARKS[mark_i]:
                    incs += 1
                    mark_i += 1
                if incs:
                    inst.then_inc(vsem, incs)

            def flush_pending(force=False):
                """Emit pending H ops whose dep chain differs from last op."""
                while pending:
                    cid, fn, comps = pending[0]
                    if cid == last_chain[0]:
                        if not force:
                            return
                        vector.engine_nop()
                        last_chain[0] = None
                    pending.pop(0)
                    inst = fn()
                    last_chain[0] = cid
                    _complete(inst, comps)

            def pair_dims(mem, inner):
                """AP dims: leading [baseB-baseA, 2] when mem has 2 rois."""
                if len(mem) == 2:
                    d0 = sbase(mem[0])
                    d1 = sbase(mem[1])
                    return [[d1 - d0, 2]] + inner
                return list(inner)

            def sbase(bn):
                b, n = bn
                return (int(y1[b, n]) + offs[b]) * W + int(x1[b, n])

            def grp_ap(mem, dr, dc, inner):
                b, n = mem[0]
                base = sl.offset + sbase(mem[0]) + dr * W + dc
                return bass.AP(slab_t, base, [part_pair] + pair_dims(mem, inner))

            tmp_i = 0
            for u in units:
                for cc in range(u["cneed"] + 1):
                    if cc not in waited:
                        vector.wait_ge(chunk_sems[cc], 16)
                        waited.add(cc)
                kind = u["kind"]
                mem = u["rois"]
                if kind == "one":
                    b, n = mem[0]
                    _kw, _sh, _sw = (
                        int(kw[b, n]), int(sh[b, n]), int(sw[b, n]))
                    m = len(mem)
                    s0 = slots[mem[0]]
                    assert [slots[bn] for bn in mem] == list(range(s0, s0 + m))
                    dst = slot_ap(s0, m)
                    inner = [[_sh * W, 7], [_sw, 7]]
                    cid = mem[-1]
                    if _kw == 1:
                        emit(cid, lambda dst=dst, mem=mem, inner=inner:
                             vector.tensor_copy(dst, grp_ap(mem, 0, 0, inner)),
                             list(mem))
                    elif _kw == 2:
                        emit(cid, lambda dst=dst, mem=mem, inner=inner:
                             vector.tensor_tensor(
                                 dst, grp_ap(mem, 0, 0, inner),
                                 grp_ap(mem, 0, 1, inner), op=AluMax),
                             list(mem))
                    else:
                        emit(cid, lambda dst=dst, mem=mem, inner=inner, _kw=_kw:
                             vector.reduce_max(
                                 dst, grp_ap(mem, 0, 0, inner + [[1, _kw]]),
                                 axis=AxisX),
                             list(mem))
                    flush_pending()
                elif kind == "chain":
                    khs = [int(kh[b, n]) for b, n in mem]
                    _sh = int(sh[mem[0][0], mem[0][1]])
                    m = len(mem)
                    s0 = slots[mem[0]]
                    assert [slots[bn] for bn in mem] == list(range(s0, s0 + m))
                    dst = slot_ap(s0, m)
                    inner = [[_sh * W, 7], [1, 7]]
                    cid = mem[-1]
                    khmin, khmax = min(khs), max(khs)
                    emit(cid, lambda dst=dst, mem=mem, inner=inner:
                         vector.tensor_tensor(
                             dst, grp_ap(mem, 0, 0, inner),
                             grp_ap(mem, 1, 0, inner), op=AluMax),
                         [bn for bn, k_ in zip(mem, khs) if k_ == 2])
                    for d in range(2, khmax):
                        cur = [bn for bn, k_ in zip(mem, khs) if k_ > d]
                        cd = dst if len(cur) == m else \
                            slot_ap(slots[cur[0]], len(cur))
                        if last_chain[0] == cid:
                            flush_pending()
                        if last_chain[0] == cid:
                            vector.engine_nop()
                            last_chain[0] = None
                        comps = [bn for bn, k_ in zip(mem, khs) if k_ == d + 1]
                        emit(cid, lambda cd=cd, cur=cur, inner=inner, d=d:
                             vector.tensor_tensor(
                                 cd, cd, grp_ap(cur, d, 0, inner), op=AluMax),
                             comps)
                    flush_pending()
                else:  # split
                    _sw, _kw = u["key"]
                    we = _wext(_sw, _kw)
                    tmp = tmps[tmp_i % 4]
                    tmp_i += 1
                    vpairs = u["vpairs"]
                    tap = tmp[:]
                    tpart = list(tap.ap[0])
                    # strip APs per vpair (pair dim over the 2 strips)
                    off_i = {}
                    idx0 = 0
                    for vp in vpairs:
                        off_i[id(vp)] = idx0
                        idx0 += len(vp)

                    def strip_ap(vp, solo_idx=None):
                        i0 = off_i[id(vp)]
                        if solo_idx is not None:
                            o = tap.offset + (i0 + solo_idx) * 7 * we
                            return bass.AP(tap.tensor, o,
                                           [tpart, [we, 7], [1, we]])
                        if len(vp) == 2:
                            return bass.AP(tap.tensor, tap.offset + i0 * 7 * we,
                                           [tpart, [7 * we, 2], [we, 7], [1, we]])
                        return bass.AP(tap.tensor, tap.offset + i0 * 7 * we,
                                       [tpart, [we, 7], [1, we]])

                    khmax_all = max(int(kh[b, n]) for b, n in mem)
                    for d in range(1, khmax_all):
                        for vp in vpairs:
                            khs = [int(kh[b, n]) for b, n in vp]
                            _shv = int(sh[vp[0][0], vp[0][1]])
                            inner = [[_shv * W, 7], [1, we]]
                            cidv = ("v",) + tuple(vp)
                            if len(vp) == 2 and d < min(khs):
                                sap = strip_ap(vp)
                                if d == 1:
                                    fn = (lambda sap=sap, vp=vp, inner=inner:
                                          vector.tensor_tensor(
                                              sap, grp_ap(vp, 0, 0, inner),
                                              grp_ap(vp, 1, 0, inner),
                                              op=AluMax))
                                else:
                                    fn = (lambda sap=sap, vp=vp, inner=inner,
                                          d=d:
                                          vector.tensor_tensor(
                                              sap, sap,
                                              grp_ap(vp, d, 0, inner),
                                              op=AluMax))
                            else:
                                # solo step for whichever members still run
                                cur = [(i3, bn) for i3, (bn, k_) in
                                       enumerate(zip(vp, khs)) if k_ > d]
                                if not cur:
                                    continue
                                assert len(cur) == 1
                                i3, bn = cur[0]
                                sap = strip_ap(vp, solo_idx=i3)
                                if d == 1:
                                    fn = (lambda sap=sap, bn=bn, inner=inner:
                                          vector.tensor_tensor(
                                              sap, grp_ap([bn], 0, 0, inner),
                                              grp_ap([bn], 1, 0, inner),
                                              op=AluMax))
                                else:
                                    fn = (lambda sap=sap, bn=bn, inner=inner,
                                          d=d:
                                          vector.tensor_tensor(
                                              sap, sap,
                                              grp_ap([bn], d, 0, inner),
                                              op=AluMax))
                            if last_chain[0] == cidv:
                                flush_pending()
                            if last_chain[0] == cidv:
                                vector.engine_nop()
                                last_chain[0] = None
                            emit(cidv, fn)
                        if d == 1:
                            flush_pending()
                    # horizontal over all strips
                    npair = len(mem)
                    s0 = min(slots[bn] for bn in mem)
                    assert [slots[bn] for bn in mem] == list(
                        range(s0, s0 + npair))
                    hcid = last_chain[0]

                    def mk_hsrc(tap_t=tap.tensor, tap_off=tap.offset,
                                tpart=tuple(tpart), we=we, sw_=_sw,
                                npair=npair):
                        def hsrc(dc, extra=None):
                            dims = ([[7 * we, npair]] if npair >= 2 else []) \
                                + [[we, 7], [sw_, 7]]
                            if extra:
                                dims = dims + [extra]
                            return bass.AP(tap_t, tap_off + dc,
                                           [list(tpart)] + dims)
                        return hsrc

                    hsrc = mk_hsrc()
                    out2 = slot_ap(s0, npair)
                    if _kw == 1:
                        pending.append((hcid, lambda out2=out2, hsrc=hsrc:
                                        vector.tensor_copy(out2, hsrc(0)),
                                        list(mem)))
                    elif _kw == 2:
                        pending.append((hcid, lambda out2=out2, hsrc=hsrc:
                                        vector.tensor_tensor(
                                            out2, hsrc(0), hsrc(1), op=AluMax),
                                        list(mem)))
                    elif _kw == 3:
                        pending.append((hcid, lambda out2=out2, hsrc=hsrc:
                                        vector.tensor_tensor(
                                            out2, hsrc(0), hsrc(1), op=AluMax),
                                        ()))
                        pending.append((hcid, lambda out2=out2, hsrc=hsrc:
                                        vector.tensor_tensor(
                                            out2, out2, hsrc(2), op=AluMax),
                                        list(mem)))
                    else:
                        pending.append((hcid, lambda out2=out2, hsrc=hsrc,
                                        _kw=_kw:
                                        vector.reduce_max(
                                            out2, hsrc(0, [1, _kw]), axis=AxisX),
                                        list(mem)))
            flush_pending(force=True)
            if mark_i < len(MARKS):
                raise RuntimeError("marks not all reached")

        @block.vector
        def _(vector):
            pid = vector.alloc_register("pid")
            vector.reg_load(pid, nc.partition_id_tensor[0:1, 0:1])

            # balanced dispatch: 2 branches deep for every core
            with vector.If_lt(pid, 4):
                with vector.If_lt(pid, 2):
                    emit_body(vector, 0)
                with vector.Else():
                    emit_body(vector, 1)
            with vector.Else():
                with vector.If_lt(pid, 6):
                    emit_body(vector, 2)
                with vector.Else():
                    emit_body(vector, 3)

    return nc, bodies, r_max


_CACHE: dict[bytes, object] = {}
LAST_RESULT = None
LAST_PLAN = None
DEBUG_OPS: dict[int, list] = {}


def _get_built(params_key: bytes, params):
    built = _CACHE.get(params_key)
    if built is None:
        built = _build_nc(params)
        _CACHE[params_key] = built
    return built


def kernel(rois: np.ndarray, conv_out: np.ndarray) -> np.ndarray:
    from concourse.bass_utils import run_bass_kernel_spmd

    rois = np.asarray(rois)
    conv_out = np.asarray(conv_out, np.float32)
    params = _roi_params(rois)
    params_key = b"".join(np.ascontiguousarray(p).tobytes() for p in params)
    nc, bodies, r_max = _get_built(params_key, params)
    global LAST_PLAN
    LAST_PLAN = (bodies, r_max)

    in_maps = []
    for core in range(N_CORES):
        j, cg = core >> 1, core & 1
        bd = bodies[j]
        slab = np.zeros((128, r_max, W), np.float16)
        for b, (lo, hi) in bd["bands"].items():
            off = bd["offs"][b] + lo
            slab[:, off : off + hi - lo + 1] = conv_out[
                b, cg * 128 : (cg + 1) * 128, lo : hi + 1, :
            ]
        in_maps.append({"conv": slab.reshape(128, -1)})

    res = run_bass_kernel_spmd(nc, in_maps, list(range(N_CORES)))
    global LAST_RESULT
    LAST_RESULT = res

    out = np.empty((B, N, C, POOL_H, POOL_W), np.float32)
    for core in range(N_CORES):
        j, cg = core >> 1, core & 1
        bd = bodies[j]
        r = (
            res.results[core]["out"]
            .reshape(128, N_PER_CORE, CELLS)
            .astype(np.float32)
        )
        for b, n in bd["rois"]:
            s = bd["slots"][(b, n)]
            out[b, n, cg * 128 : (cg + 1) * 128] = r[:, s].reshape(
                128, POOL_H, POOL_W
            )
    return out


# revision 51
# speedup vs baseline: 1.0526x; 1.0423x over previous
"""ROI max-pooling (B=2, N=64, C=256, H=W=64, 7x7 out) on 8 TRN2 cores — v4.

Design (v4):
- Host converts conv_out to fp16 and pre-slices a per-core row band
  (uniform R_MAX rows across cores); core = b*4 + nh*2 + cg.
- All reduction on the DVE, with forms chosen for the DVE perf modes:
  * TensorTensor max gets 2x_1p (0.5 cyc/elem) when all operands are
    fp16 with a packed (stride-1, count>=2) innermost AP dim.
  * TensorCopy gets 2x_2p (SBUF) / 4x when also packed.
  * TensorReduce gets NO perf mode (1 cyc/elem) — avoid where a TT
    chain is cheaper.
  Forms:
  * kh=1,kw=1: single 4x copy slab->ostage.
  * kh=1,kw=2: single TT from slab.
  * kh=1,kw>=3: single reduce from slab.
  * kh>=2,kw=1 (sw==1 always in-dist): TT chain writing ostage
    directly (wext==7), no tmp, no horizontal stage.
  * kh>=2,kw>=2: vertical TT chain into a [7,wext] fp16 strip (always
    2x), then a horizontal stage from the strip. Horizontal stages of
    two ROIs with the same (sw,kw) are PAIRED: both strips go in one
    tmp and one instruction covers both output slots.
- Same-engine RAW chains (vertical steps, chained horizontals) are
  spaced by interleaving the two chains of a pair and by flushing the
  previous pair's horizontal into the gap; engine_nop as last resort.
- Output staged fp16, slot order = completion order, slice marks are
  uniform across bodies so SP/Act need no registers or branches.
"""

import os

os.environ.setdefault("MYCRO_LOCAL_CACHE", "1")

import numpy as np

B, N, C, H, W = 2, 64, 256, 64, 64
POOL_H = POOL_W = 7
ANCHOR_STRIDE = 16
N_CORES = 8
N_PER_CORE = N // 2  # 32
CELLS = POOL_H * POOL_W  # 49
N_CHUNKS = 4

# ---- DVE cost model (ns), calibrated from HW traces ------------------------
F_OP = 72.0  # per-instruction fixed busy (TT/copy; reduce ~62)
R1 = 1.042  # cyc/elem, no perf mode (reduce, strided TT)
R2 = 0.53  # 2x_1p TT / 2x_2p copy
R4 = 0.53  # copy (4x not observed on HW; treat as 2x)

# schedule model constants (ns), relative to NEFF t=0
_ROW_NS = 108.0  # fp16 row (128p x 64 x 2B = 16 KiB) per-queue transfer
_T_SEM = 1050.0  # completion sem propagation + wake
_T_SETUP = 10600.0  # vector regs + branch resolved (abs)


def _wext(sw, kw):
    return 6 * sw + kw


def _roi_cost(kh, kw, sh, sw):
    """(cost_ns, n_ops) for one ROI, horizontal unpaired."""
    if kh == 1:
        if kw == 1:
            return (F_OP + R4 * 49, 1)
        if kw == 2:
            r = R2 if sw == 1 else R1
            return (F_OP + r * 49, 1)
        return (F_OP + R1 * 49 * kw, 1)
    if kw == 1:
        return ((kh - 1) * (F_OP + R2 * 49), kh - 1)
    we = _wext(sw, kw)
    v = (kh - 1) * (F_OP + R2 * 7 * we)
    h, nh = _h_cost(sw, kw, 1)
    return (v + h, kh - 1 + nh)


def _h_cost(sw, kw, npair):
    """(cost_ns, n_ops) of the horizontal stage covering npair strips."""
    e = 49 * npair
    if kw == 1:
        return (F_OP + R2 * e, 1)  # strided copy, 2x_2p
    if kw == 2:
        r = R2 if sw == 1 else R1
        return (F_OP + r * e, 1)
    if kw == 3:
        return (2 * (F_OP + R1 * e), 2)  # TT chain
    return (F_OP + R1 * e * kw, 1)  # kw=4: reduce


# ---- roi params ------------------------------------------------------------
def _expand(lo, hi, pool, limit):
    for _ in range(pool):
        need = (hi - lo + 1) < pool
        lo = np.where(need, np.maximum(0, lo - 1), lo)
        hi = np.where(need, np.minimum(limit - 1, hi + 1), hi)
    return lo, hi


def _roi_params(rois: np.ndarray):
    coords = (np.asarray(rois, np.float32) / ANCHOR_STRIDE).astype(np.int32)
    x1, y1, x2, y2 = (coords[..., i] for i in range(4))
    y1, y2 = _expand(y1, y2, POOL_H, H)
    x1, x2 = _expand(x1, x2, POOL_W, W)
    rh = y2 - y1 + 1
    rw = x2 - x1 + 1
    kh = -(-rh // POOL_H)
    sh = rh // POOL_H
    kw = -(-rw // POOL_W)
    sw = rw // POOL_W
    return y1, x1, sh, sw, kh, kw


# ---- planning --------------------------------------------------------------
def _row_extent(params, b, n):
    y1, x1, sh, sw, kh, kw = params
    lo = int(y1[b, n])
    hi = lo + 6 * int(sh[b, n]) + int(kh[b, n]) - 1
    return lo, hi


def _chunk_bounds(r_max, c0):
    rest = r_max - c0
    s1 = max(10, rest // 3)
    s2 = (rest - s1) // 2
    sizes = [c0, s1, s2, rest - s1 - s2]
    bounds = []
    acc = 0
    for s in sizes:
        acc += s
        bounds.append(acc)
    return bounds


def _c0_for(params, rs, metas, r_max):
    """Chunk-0 rows: enough that every body has >=1 ROI resident."""
    need = 0
    for rois, (bands, offs, rows) in zip(rs, metas):
        phis = sorted(
            offs[b] + _row_extent(params, b, n)[1] for b, n in rois
        )
        need = max(need, phis[0] + 1)
    return max(4, min(need, r_max // 2))


def _land_times(r_max, c0):
    """Modeled absolute landing time per chunk.
    q0 carries c0,c2 (first bytes ~10.1us); q1 carries c1,c3 (~10.6us)."""
    b = _chunk_bounds(r_max, c0)
    s = [b[0]] + [b[i] - b[i - 1] for i in range(1, N_CHUNKS)]
    return [
        10100 + s[0] * _ROW_NS + _T_SEM,
        10600 + s[1] * _ROW_NS + _T_SEM,
        10100 + (s[0] + s[2]) * _ROW_NS + _T_SEM,
        10600 + (s[1] + s[3]) * _ROW_NS + _T_SEM,
    ]


def _chunk_of(row, r_max, c0):
    bounds = _chunk_bounds(r_max, c0)
    rel = min(max(row, 0), r_max - 1)
    for c, e in enumerate(bounds):
        if rel < e:
            return c
    return N_CHUNKS - 1


def _bands(params, rois):
    bands = {}
    for b, n in rois:
        lo, hi = _row_extent(params, b, n)
        if b in bands:
            bands[b] = (min(bands[b][0], lo), max(bands[b][1], hi))
        else:
            bands[b] = (lo, hi)
    rows = 0
    offs = {}
    for b in sorted(bands):
        offs[b] = rows - bands[b][0]
        rows += bands[b][1] - bands[b][0] + 1
    return bands, offs, rows


def _units_for(params, rois, offs, r_max, c0):
    """Build scheduling units.

    Split ROIs (kh>=2, kw>=2) form units of 1-4 members grouped as
    "vpairs": two members with the same (sh, sw, kw) share each vertical
    TT via an extra [baseB-baseA, 2] AP dim; a unit's members share one
    horizontal op (same (sw, kw)). kw=1 chains and kh=1 ones pair the
    same way (shared ops with a pair dim on both src and dst).
    """
    y1, x1, sh, sw, kh, kw = params

    def cneed(bn):
        b, n = bn
        lo, hi = _row_extent(params, b, n)
        return _chunk_of(offs[b] + hi, r_max, c0)

    splits, chains, ones = [], {}, {}
    sgroups = {}
    for bn in rois:
        b, n = bn
        _kh, _kw, _sh, _sw = (
            int(kh[b, n]), int(kw[b, n]), int(sh[b, n]), int(sw[b, n]))
        if _kh >= 2 and _kw >= 2:
            sgroups.setdefault((_sh, _sw, _kw), []).append(bn)
        elif _kh >= 2:
            chains.setdefault(_sh, []).append(bn)
        else:
            ones.setdefault((_kw, _sh, _sw), []).append(bn)

    units = []

    # split rois: same-(sh,sw,kw) vpairs, then vpairs merged into units
    # of up to 4 rois with the same (sw,kw)
    vpairs = {}
    for (s_, w_, k_), mem in sgroups.items():
        mem.sort(key=lambda bn: (int(kh[bn[0], bn[1]]), cneed(bn)))
        i = 0
        while i < len(mem):
            if i + 1 < len(mem) and cneed(mem[i + 1]) <= cneed(mem[i]) + 1:
                vpairs.setdefault((w_, k_), []).append(mem[i : i + 2])
                i += 2
            else:
                vpairs.setdefault((w_, k_), []).append([mem[i]])
                i += 1
    for key, vps in vpairs.items():
        vps.sort(key=lambda vp: max(cneed(bn) for bn in vp))
        i = 0
        while i < len(vps):
            if (
                i + 1 < len(vps)
                and len(vps[i]) + len(vps[i + 1]) <= 4
                and max(cneed(bn) for bn in vps[i + 1])
                <= max(cneed(bn) for bn in vps[i]) + 1
            ):
                grp = [vps[i], vps[i + 1]]
                i += 2
            else:
                grp = [vps[i]]
                i += 1
            units.append(dict(kind="split", vpairs=grp, key=key,
                              rois=[bn for vp in grp for bn in vp]))

    # kw=1 chains: pair by same sh (kh may differ -> tail steps)
    for s_, mem in chains.items():
        mem.sort(key=lambda bn: (int(kh[bn[0], bn[1]]), cneed(bn)))
        i = 0
        while i < len(mem):
            if i + 1 < len(mem) and cneed(mem[i + 1]) <= cneed(mem[i]) + 1:
                units.append(dict(kind="chain", rois=mem[i : i + 2]))
                i += 2
            else:
                units.append(dict(kind="chain", rois=[mem[i]]))
                i += 1

    # kh=1 ones: pair by same (kw,sh,sw) -> one shared op
    for key1, mem in ones.items():
        mem.sort(key=cneed)
        i = 0
        while i < len(mem):
            if i + 1 < len(mem) and cneed(mem[i + 1]) <= cneed(mem[i]) + 1:
                units.append(dict(kind="one", rois=mem[i : i + 2]))
                i += 2
            else:
                units.append(dict(kind="one", rois=[mem[i]]))
                i += 1

    def sbase(bn):
        b, n = bn
        return (int(y1[b, n]) + offs[b]) * W + int(x1[b, n])

    for u in units:
        if u["kind"] == "split":
            for vp in u["vpairs"]:
                vp.sort(key=sbase)
            u["rois"] = [bn for vp in u["vpairs"] for bn in vp]
        else:
            u["rois"].sort(key=sbase)

    for u in units:
        u["cneed"] = max(cneed(bn) for bn in u["rois"])
        cost = 0.0
        if u["kind"] == "split":
            _sw, _kw = u["key"]
            we = _wext(_sw, _kw)
            for vp in u["vpairs"]:
                khs = sorted(int(kh[b, n]) for b, n in vp)
                if len(vp) == 2:
                    shared = khs[0] - 1
                    tail = khs[1] - khs[0]
                    cost += shared * (F_OP + R2 * 14 * we)
                    cost += tail * (F_OP + R2 * 7 * we)
                else:
                    cost += (khs[0] - 1) * (F_OP + R2 * 7 * we)
            cost += _h_cost(_sw, _kw, len(u["rois"]))[0]
        elif u["kind"] == "chain":
            khs = sorted(int(kh[b, n]) for b, n in u["rois"])
            if len(u["rois"]) == 2:
                cost += (khs[0] - 1) * (F_OP + R2 * 98)
                cost += (khs[1] - khs[0]) * (F_OP + R2 * 49)
            else:
                cost += (khs[0] - 1) * (F_OP + R2 * 49)
        else:
            b, n = u["rois"][0]
            _kw, _sw = int(kw[b, n]), int(sw[b, n])
            m = len(u["rois"])
            if _kw == 1:
                cost += F_OP + R4 * 49 * m
            elif _kw == 2:
                r = R2 if _sw == 1 else R1
                cost += F_OP + r * 49 * m
            else:
                cost += F_OP + R1 * 49 * _kw * m
        u["cost"] = cost
    return units


def _sched_units(units, r_max, c0):
    """Order units by chunk readiness; return (makespan, ordered units)."""
    land = _land_times(r_max, c0)
    units = sorted(units, key=lambda u: (u["cneed"], -u["cost"]))
    clk = _T_SETUP
    for u in units:
        clk = max(clk, land[u["cneed"]]) + u["cost"]
    return clk, units


def _plan(params):
    bodies_rois = []
    for b in range(B):
        ext = [_row_extent(params, b, n) for n in range(N)]
        order = sorted(range(N), key=lambda n: ext[n][0] + ext[n][1])
        bodies_rois.append([(b, n) for n in order[:N_PER_CORE]])
        bodies_rois.append([(b, n) for n in order[N_PER_CORE:]])

    def score(rs):
        metas = [_bands(params, r) for r in rs]
        r_used = max(m[2] for m in metas)
        r_used = min(2 * H, -(-r_used // 4) * 4)
        c0 = _c0_for(params, rs, metas, r_used)
        mks = []
        for r, (bands, offs, rows) in zip(rs, metas):
            units = _units_for(params, r, offs, r_used, c0)
            mk, _ = _sched_units(units, r_used, c0)
            mks.append(mk)
        return max(mks) + 3.0 * r_used + 8.0 * c0

    base_rois = [list(r) for r in bodies_rois]
    best_rois, best_score = None, None
    for seed in range(3):
        bodies_rois = [list(r) for r in base_rois]
        cur = score(bodies_rois)
        rng = np.random.default_rng(seed)
        pairs = [(0, 1), (2, 3)] * 6 + [(0, 2), (1, 3), (0, 3), (1, 2)]
        for it in range(2500):
            if it % 3 < 2:
                j1, j2 = pairs[int(rng.integers(0, len(pairs)))]
            else:
                j1, j2 = int(rng.integers(0, 4)), int(rng.integers(0, 4))
                if j1 == j2:
                    continue
            i1 = int(rng.integers(0, N_PER_CORE))
            i2 = int(rng.integers(0, N_PER_CORE))
            a, bq = bodies_rois[j1], bodies_rois[j2]
            a[i1], bq[i2] = bq[i2], a[i1]
            new = score(bodies_rois)
            if new <= cur:
                cur = new
            else:
                a[i1], bq[i2] = bq[i2], a[i1]
        if best_score is None or cur < best_score:
            best_rois, best_score = [list(r) for r in bodies_rois], cur
    bodies_rois = best_rois

    metas = [_bands(params, r) for r in bodies_rois]
    r_max = max(m[2] for m in metas)
    r_max = min(2 * H, -(-r_max // 4) * 4)
    c0 = _c0_for(params, bodies_rois, metas, r_max)

    bodies = []
    for j in range(4):
        rois = bodies_rois[j]
        bands, offs, rows = metas[j]
        units = _units_for(params, rois, offs, r_max, c0)
        mk, order = _sched_units(units, r_max, c0)
        # slots in completion order
        slots = {}
        s = 0
        for u in order:
            for bn in u["rois"]:
                slots[bn] = s
                s += 1
        bodies.append(
            dict(rois=rois, bands=bands, offs=offs, units=order, slots=slots, mk=mk)
        )
    return bodies, r_max, c0


# ---- device program --------------------------------------------------------
MARKS = [10, 20, 27, 32]  # uniform completion-count slice marks


def _build_nc(params):
    import contextlib

    import concourse.bass as bass
    from concourse import mybir

    y1, x1, sh, sw, kh, kw = params
    f16 = mybir.dt.float16

    bodies, r_max, c0 = _plan(params)
    FS = r_max * W
    OS = N_PER_CORE * CELLS
    bounds = _chunk_bounds(r_max, c0)
    starts = [0] + bounds[:-1]

    branch_order = sorted(range(4), key=lambda j: -bodies[j]["mk"])

    nc = bass.Bass(monotonic_sem_count=0)
    conv = nc.declare_dram_parameter("conv", [128, FS], f16, isOutput=False)
    out = nc.declare_dram_parameter("out", [128, OS], f16, isOutput=True)

    with contextlib.ExitStack() as ctx:
        slab = ctx.enter_context(nc.sbuf_tensor("slab", [128, FS], f16))
        ostage = ctx.enter_context(nc.sbuf_tensor("ostage", [128, OS], f16))
        tmps = [
            ctx.enter_context(nc.sbuf_tensor(f"tmp{i}", [128, 4 * 7 * 22], f16))
            for i in range(4)
        ]
        chunk_sems = [
            ctx.enter_context(nc.semaphore(f"chunk{c}")) for c in range(N_CHUNKS)
        ]
        vsem = ctx.enter_context(nc.semaphore("vsem"))
        osem = ctx.enter_context(nc.semaphore("osem"))
        block = ctx.enter_context(nc.Block())

        sl = slab[:]
        slab_t = sl.tensor
        part_pair = list(sl.ap[0])

        def chunk_dma(eng, c):
            eng.dma_start(
                slab[:, starts[c] * W : bounds[c] * W],
                conv[:, starts[c] * W : bounds[c] * W],
            ).then_inc(chunk_sems[c], 16)

        def out_slice(eng, lo_s, hi_s, thresh):
            eng.wait_ge(vsem, thresh)
            eng.dma_start(
                out[:, lo_s * CELLS : hi_s * CELLS],
                ostage[:, lo_s * CELLS : hi_s * CELLS],
            ).then_inc(osem, 16)

        @block.sync
        def _(sync):
            chunk_dma(sync, 0)
            chunk_dma(sync, 2)
            out_slice(sync, 0, MARKS[0], 1)
            out_slice(sync, MARKS[1], MARKS[2], 3)

        @block.scalar
        def _(scalar):
            chunk_dma(scalar, 1)
            chunk_dma(scalar, 3)
            out_slice(scalar, MARKS[0], MARKS[1], 2)
            out_slice(scalar, MARKS[2], MARKS[3], 4)

        AluMax = mybir.AluOpType.max
        AxisX = mybir.AxisListType.X

        def emit_body(vector, j):
            # wrap compute methods to log (op, free_elems, packed) per emission
            dbg = DEBUG_OPS.setdefault(j, [])

            def _packed(ap):
                try:
                    last = ap.ap[-1]
                    return last[0] in (1, -1) and last[1] >= 2
                except Exception:
                    return False

            def _fs(ap):
                fs = 1
                for st, ct in list(ap.ap)[1:]:
                    fs *= ct
                return fs

            _tt, _rm, _tc, _nop = (vector.tensor_tensor, vector.reduce_max,
                                   vector.tensor_copy, vector.engine_nop)
            _dma = vector.dma_start

            def tt(out, a, b_, **kw_):
                dbg.append(("tt", max(_fs(a), _fs(b_), _fs(out)),
                            _packed(a) and _packed(b_) and _packed(out)))
                return _tt(out, a, b_, **kw_)

            def rm(out, in_, **kw_):
                dbg.append(("red", max(_fs(in_), _fs(out)), False))
                return _rm(out, in_, **kw_)

            def tc(out, in_):
                dbg.append(("copy", max(_fs(in_), _fs(out)),
                            _packed(in_) and _packed(out)))
                return _tc(out, in_)

            def nop():
                dbg.append(("nop", 0, False))
                return _nop()

            vector = type("V", (), dict(
                tensor_tensor=staticmethod(tt), reduce_max=staticmethod(rm),
                tensor_copy=staticmethod(tc), engine_nop=staticmethod(nop),
                dma_start=staticmethod(_dma),
                wait_ge=staticmethod(vector.wait_ge)))()

            bd = bodies[j]
            offs = bd["offs"]
            slots = bd["slots"]
            units = bd["units"]

            def slab_ap(b, n, dr, dc, inner):
                base = (
                    sl.offset
                    + (int(y1[b, n]) + offs[b] + dr) * W
                    + int(x1[b, n])
                    + dc
                )
                return bass.AP(slab_t, base, [part_pair] + inner)

            def slot_ap(s, count=1):
                return ostage[:, s * CELLS : (s + count) * CELLS]

            waited = set()
            done = 0
            mark_i = 0
            last_chain = [None]  # chain id of previously emitted op
            pending = []  # list of (chain_id, emit_fn, completes)

            def emit(chain_id, fn, completes=()):
                """Emit one op; flush pending H ops into gaps."""
                nonlocal done, mark_i
                inst = fn()
                last_chain[0] = chain_id
                _complete(inst, completes)
                return inst

            def _complete(inst, completes):
                nonlocal done, mark_i
                if not completes:
                    return
                done += len(completes)
                incs = 0
                while mark_i < len(MARKS) and done >= MARKS[mark_i]:
                    incs += 1
                    mark_i += 1
                if incs:
                    inst.then_inc(vsem, incs)

            def flush_pending(force=False):
                """Emit pending H ops whose dep chain differs from last op."""
                while pending:
                    cid, fn, comps = pending[0]
                    if cid == last_chain[0]:
                        if not force:
                            return
                        vector.engine_nop()
                        last_chain[0] = None
                    pending.pop(0)
                    inst = fn()
                    last_chain[0] = cid
                    _complete(inst, comps)

            def pair_dims(mem, inner):
                """AP dims: leading [baseB-baseA, 2] when mem has 2 rois."""
                if len(mem) == 2:
                    d0 = sbase(mem[0])
                    d1 = sbase(mem[1])
                    return [[d1 - d0, 2]] + inner
                return list(inner)

            def sbase(bn):
                b, n = bn
                return (int(y1[b, n]) + offs[b]) * W + int(x1[b, n])

            def grp_ap(mem, dr, dc, inner):
                b, n = mem[0]
                base = sl.offset + sbase(mem[0]) + dr * W + dc
                return bass.AP(slab_t, base, [part_pair] + pair_dims(mem, inner))

            tmp_i = 0
            for u in units:
                for cc in range(u["cneed"] + 1):
                    if cc not in waited:
                        vector.wait_ge(chunk_sems[cc], 16)
                        waited.add(cc)
                kind = u["kind"]
                mem = u["rois"]
                if kind == "one":
                    b, n = mem[0]
                    _kw, _sh, _sw = (
                        int(kw[b, n]), int(sh[b, n]), int(sw[b, n]))
                    m = len(mem)
                    s0 = slots[mem[0]]
                    assert [slots[bn] for bn in mem] == list(range(s0, s0 + m))
                    dst = slot_ap(s0, m)
                    inner = [[_sh * W, 7], [_sw, 7]]
                    cid = mem[-1]
                    if _kw == 1:
                        emit(cid, lambda dst=dst, mem=mem, inner=inner:
                             vector.tensor_copy(dst, grp_ap(mem, 0, 0, inner)),
                             list(mem))
                    elif _kw == 2:
                        emit(cid, lambda dst=dst, mem=mem, inner=inner:
                             vector.tensor_tensor(
                                 dst, grp_ap(mem, 0, 0, inner),
                                 grp_ap(mem, 0, 1, inner), op=AluMax),
                             list(mem))
                    else:
                        emit(cid, lambda dst=dst, mem=mem, inner=inner, _kw=_kw:
                             vector.reduce_max(
                                 dst, grp_ap(mem, 0, 0, inner + [[1, _kw]]),
                                 axis=AxisX),
                             list(mem))
                    flush_pending()
                elif kind == "chain":
                    khs = [int(kh[b, n]) for b, n in mem]
                    _sh = int(sh[mem[0][0], mem[0][1]])
                    m = len(mem)
                    s0 = slots[mem[0]]
                    assert [slots[bn] for bn in mem] == list(range(s0, s0 + m))
                    dst = slot_ap(s0, m)
                    inner = [[_sh * W, 7], [1, 7]]
                    cid = mem[-1]
                    khmin, khmax = min(khs), max(khs)
                    emit(cid, lambda dst=dst, mem=mem, inner=inner:
                         vector.tensor_tensor(
                             dst, grp_ap(mem, 0, 0, inner),
                             grp_ap(mem, 1, 0, inner), op=AluMax),
                         [bn for bn, k_ in zip(mem, khs) if k_ == 2])
                    for d in range(2, khmax):
                        cur = [bn for bn, k_ in zip(mem, khs) if k_ > d]
                        cd = dst if len(cur) == m else \
                            slot_ap(slots[cur[0]], len(cur))
                        if last_chain[0] == cid:
                            flush_pending()
                        if last_chain[0] == cid:
                            vector.engine_nop()
                            last_chain[0] = None
                        comps = [bn for bn, k_ in zip(mem, khs) if k_ == d + 1]
                        emit(cid, lambda cd=cd, cur=cur, inner=inner, d=d:
                             vector.tensor_tensor(
                                 cd, cd, grp_ap(cur, d, 0, inner), op=AluMax),
                             comps)
                    flush_pending()
                else:  # split
                    _sw, _kw = u["key"]
                    we = _wext(_sw, _kw)
                    tmp = tmps[tmp_i % 4]
                    tmp_i += 1
                    vpairs = u["vpairs"]
                    tap = tmp[:]
                    tpart = list(tap.ap[0])
                    # strip APs per vpair (pair dim over the 2 strips)
                    off_i = {}
                    idx0 = 0
                    for vp in vpairs:
                        off_i[id(vp)] = idx0
                        idx0 += len(vp)

                    def strip_ap(vp, solo_idx=None):
                        i0 = off_i[id(vp)]
                        if solo_idx is not None:
                            o = tap.offset + (i0 + solo_idx) * 7 * we
                            return bass.AP(tap.tensor, o,
                                           [tpart, [we, 7], [1, we]])
                        if len(vp) == 2:
                            return bass.AP(tap.tensor, tap.offset + i0 * 7 * we,
                                           [tpart, [7 * we, 2], [we, 7], [1, we]])
                        return bass.AP(tap.tensor, tap.offset + i0 * 7 * we,
                                       [tpart, [we, 7], [1, we]])

                    khmax_all = max(int(kh[b, n]) for b, n in mem)
                    for d in range(1, khmax_all):
                        for vp in vpairs:
                            khs = [int(kh[b, n]) for b, n in vp]
                            _shv = int(sh[vp[0][0], vp[0][1]])
                            inner = [[_shv * W, 7], [1, we]]
                            cidv = ("v",) + tuple(vp)
                            if len(vp) == 2 and d < min(khs):
                                sap = strip_ap(vp)
                                if d == 1:
                                    fn = (lambda sap=sap, vp=vp, inner=inner:
                                          vector.tensor_tensor(
                                              sap, grp_ap(vp, 0, 0, inner),
                                              grp_ap(vp, 1, 0, inner),
                                              op=AluMax))
                                else:
                                    fn = (lambda sap=sap, vp=vp, inner=inner,
                                          d=d:
                                          vector.tensor_tensor(
                                              sap, sap,
                                              grp_ap(vp, d, 0, inner),
                                              op=AluMax))
                            else:
                                # solo step for whichever members still run
                                cur = [(i3, bn) for i3, (bn, k_) in
                                       enumerate(zip(vp, khs)) if k_ > d]
                                if not cur:
                                    continue
                                assert len(cur) == 1
                                i3, bn = cur[0]
                                sap = strip_ap(vp, solo_idx=i3)
                                if d == 1:
                                    fn = (lambda sap=sap, bn=bn, inner=inner:
                                          vector.tensor_tensor(
                                              sap, grp_ap([bn], 0, 0, inner),
                                              grp_ap([bn], 1, 0, inner),
                                              op=AluMax))
                                else:
                                    fn = (lambda sap=sap, bn=bn, inner=inner,
                                          d=d:
                                          vector.tensor_tensor(
                                              sap, sap,
                                              grp_ap([bn], d, 0, inner),
                                              op=AluMax))
                            if last_chain[0] == cidv:
                                flush_pending()
                            if last_chain[0] == cidv:
                                vector.engine_nop()
                                last_chain[0] = None
                            emit(cidv, fn)
                        if d == 1:
                            flush_pending()
                    # horizontal over all strips
                    npair = len(mem)
                    s0 = min(slots[bn] for bn in mem)
                    assert [slots[bn] for bn in mem] == list(
                        range(s0, s0 + npair))
                    hcid = last_chain[0]

                    def mk_hsrc(tap_t=tap.tensor, tap_off=tap.offset,
                                tpart=tuple(tpart), we=we, sw_=_sw,
                                npair=npair):
                        def hsrc(dc, extra=None):
                            dims = ([[7 * we, npair]] if npair >= 2 else []) \
                                + [[we, 7], [sw_, 7]]
                            if extra:
                                dims = dims + [extra]
                            return bass.AP(tap_t, tap_off + dc,
                                           [list(tpart)] + dims)
                        return hsrc

                    hsrc = mk_hsrc()
                    out2 = slot_ap(s0, npair)
                    if _kw == 1:
                        pending.append((hcid, lambda out2=out2, hsrc=hsrc:
                                        vector.tensor_copy(out2, hsrc(0)),
                                        list(mem)))
                    elif _kw == 2:
                        pending.append((hcid, lambda out2=out2, hsrc=hsrc:
                                        vector.tensor_tensor(
                                            out2, hsrc(0), hsrc(1), op=AluMax),
                                        list(mem)))
                    elif _kw == 3:
                        pending.append((hcid, lambda out2=out2, hsrc=hsrc:
                                        vector.tensor_tensor(
                                            out2, hsrc(0), hsrc(1), op=AluMax),
                                        ()))
                        pending.append((hcid, lambda out2=out2, hsrc=hsrc:
                                        vector.tensor_tensor(
                                            out2, out2, hsrc(2), op=AluMax),
                                        list(mem)))
                    else:
                        pending.append((hcid, lambda out2=out2, hsrc=hsrc,
                                        _kw=_kw:
                                        vector.reduce_max(
                                            out2, hsrc(0, [1, _kw]), axis=AxisX),
                                        list(mem)))
            flush_pending(force=True)
            if mark_i < len(MARKS):
                raise RuntimeError("marks not all reached")

        @block.vector
        def _(vector):
            pid = vector.alloc_register("pid")
            vector.reg_load(pid, nc.partition_id_tensor[0:1, 0:1])

            # balanced dispatch: 2 branches deep for every core
            with vector.If_lt(pid, 4):
                with vector.If_lt(pid, 2):
                    emit_body(vector, 0)
                with vector.Else():
                    emit_body(vector, 1)
            with vector.Else():
                with vector.If_lt(pid, 6):
                    emit_body(vector, 2)
                with vector.Else():
                    emit_body(vector, 3)

    return nc, bodies, r_max


_CACHE: dict[bytes, object] = {}
LAST_RESULT = None
LAST_PLAN = None
DEBUG_OPS: dict[int, list] = {}


def _get_built(params_key: bytes, params):
    built = _CACHE.get(params_key)
    if built is None:
        built = _build_nc(params)
        _CACHE[params_key] = built
    return built


def kernel(rois: np.ndarray, conv_out: np.ndarray) -> np.ndarray:
    from concourse.bass_utils import run_bass_kernel_spmd

    rois = np.asarray(rois)
    conv_out = np.asarray(conv_out, np.float32)
    params = _roi_params(rois)
    params_key = b"".join(np.ascontiguousarray(p).tobytes() for p in params)
    nc, bodies, r_max = _get_built(params_key, params)
    global LAST_PLAN
    LAST_PLAN = (bodies, r_max)

    in_maps = []
    for core in range(N_CORES):
        j, cg = core >> 1, core & 1
        bd = bodies[j]
        slab = np.zeros((128, r_max, W), np.float16)
        for b, (lo, hi) in bd["bands"].items():
            off = bd["offs"][b] + lo
            slab[:, off : off + hi - lo + 1] = conv_out[
                b, cg * 128 : (cg + 1) * 128, lo : hi + 1, :
            ]
        in_maps.append({"conv": slab.reshape(128, -1)})

    res = run_bass_kernel_spmd(nc, in_maps, list(range(N_CORES)))
    global LAST_RESULT
    LAST_RESULT = res

    out = np.empty((B, N, C, POOL_H, POOL_W), np.float32)
    for core in range(N_CORES):
        j, cg = core >> 1, core & 1
        bd = bodies[j]
        r = (
            res.results[core]["out"]
            .reshape(128, N_PER_CORE, CELLS)
            .astype(np.float32)
        )
        for b, n in bd["rois"]:
            s = bd["slots"][(b, n)]
            out[b, n, cg * 128 : (cg + 1) * 128] = r[:, s].reshape(
                128, POOL_H, POOL_W
            )
    return out
